# revision 41
# baseline (speedup 1.0000x reference)
"""Discrete Hawkes conditional-intensity kernel for 8 Trainium2 NeuronCores.

Math
----
Reference computes, per query i with (t, s) = (t_i, s_i):

    lam_i = clip(mu[s] + alpha[s, s] * b * F[t, s], 1e-5)
    F[t, s] = sum_{tp < t} obs[tp, s] * exp(-b * (t - tp))

F obeys F[t+1] = e * (F[t] + obs[t]), e = exp(-b), i.e. it is an
exponentially-decayed prefix sum over time.  On device we build the full
table G[t, s] = mu[s] + alpha[s,s]*b*F[t, s] with a blocked formulation
(time blocks of 128 on the PE array + a 32-step cross-block carry), store
it to DRAM, and answer the 8192 queries per core with a split gather
(vector-engine one-hot selects from SBUF + indirect-DMA spill).

The gather is the crux: every DMA-based path for data-dependent
addressing costs ~1us of serial GPSIMD per 128 elements (SWDGE fixed
overhead; dma_gather ucode ~7ns/desc, ap_gather ~27ns/idx are no
better), which walls a pure indirect-DMA gather at ~70us/core.  So most
queries never touch a DMA: each G chunk tile [128, 512] is split into 8
windows of 64, and the first query of every (chunk, window, partition)
cell (~6430 of 8192) is answered on the vector engine by a prebuilt
one-hot mask + multiply + window-reduce straight from SBUF, pipelined
behind the chunk's matmuls.  Only cell-overflow queries (~1780) use
staged indirect-DMA columns against the DRAM copy of G.  Output slots
are partition-major so the result stores are contiguous per partition
(a transposed store pattern costs 16K four-byte descriptors, ~45us).

Sharding: queries (t, s) are split 8x8192 across cores (data parallel);
obs / mu / alpha / beta are replicated.  No collectives needed.
"""

import os
import sys

import numpy as np

_REPO_CANDIDATES = ("/opt/trn_rl_repo", os.path.expanduser("~/.axon_site/_ro/trn_rl_repo"))
for _p in _REPO_CANDIDATES:
    if os.path.isdir(_p) and _p not in sys.path:
        sys.path.append(_p)

import concourse.bass as bass
import concourse.tile as tile
from concourse import bacc, mybir
from concourse.bass_utils import run_bass_kernel_spmd

# Problem constants (hardcoded per spec).
N_TIME = 4096
N_SPACE = 256
BATCH = 65536
N_CORES = 8
LAM_MIN = 1e-5

P = 128               # partitions / time-block size
J = N_TIME // P       # 32 time blocks
PER_CORE = BATCH // N_CORES   # 8192 queries per core
CH = 512              # matmul N-chunk (one PSUM bank)
NCH = (J * N_SPACE) // CH     # 16 chunks over the (j, s) flat axis

# Gather layout.  Primary path: the G chunk tile [128, 512] is split into
# 8 windows of 64; each (chunk, window, partition) cell serves at most ONE
# query via a one-hot mask + multiply + reduce on the vector engine,
# straight from SBUF (~160 scanned elems per served query vs ~1.1us of
# serial GPSIMD per 128 queries for indirect DMA).  The one-hot masks for
# all 16 chunks are prebuilt in a single DVE op during the obs load.
# Cells with 2+ queries (~1780 of 8192) spill to NSP indirect-DMA columns
# gathered from the DRAM copy of G, staged by t so they fire as the table
# lands.
NWIN = 8              # 64-elem windows per chunk tile
EWIN = CH // NWIN     # 64
NSP = 16              # spill columns (128 queries each)
FQ2 = NCH * NWIN + NSP          # 144 output columns per partition
NSLOT = P * FQ2                 # 18432 slots; flat = p*FQ2 + col
# spill column f may only hold queries with t < 256*(SP_BOUND[f]+1); one
# chunk of lookahead keeps the sorted greedy fill from starving.
SP_BOUND = [min(15, f + 1) for f in range(NSP)]

f32 = mybir.dt.float32
bf16 = mybir.dt.bfloat16
i32 = mybir.dt.int32
Alu = mybir.AluOpType
Act = mybir.ActivationFunctionType


def build_nc():
    nc = bacc.Bacc("TRN2", target_bir_lowering=False, debug=False)

    gidx_h = nc.dram_tensor("gidx", [NSP * P], i32, kind="ExternalInput")
    phiw_h = nc.dram_tensor("phiw", [P, NCH * NWIN], i32, kind="ExternalInput")
    obs_h = nc.dram_tensor("obs", [N_TIME, N_SPACE], i32, kind="ExternalInput")
    mu_h = nc.dram_tensor("mu", [N_SPACE], f32, kind="ExternalInput")
    alpha_h = nc.dram_tensor("alpha", [N_SPACE, N_SPACE], f32, kind="ExternalInput")
    beta_h = nc.dram_tensor("beta", [1], f32, kind="ExternalInput")
    g_h = nc.dram_tensor("gtab", [N_TIME * N_SPACE + 2], f32, kind="Internal")
    out_h = nc.dram_tensor("out", [NSLOT], f32, kind="ExternalOutput")

    from contextlib import ExitStack

    with tile.TileContext(nc) as tc, ExitStack() as ctx:
        sb = ctx.enter_context(tc.tile_pool(name="sb", bufs=1))
        ps = ctx.enter_context(tc.tile_pool(name="ps", bufs=4, space="PSUM"))
        psr = ctx.enter_context(tc.tile_pool(name="psr", bufs=2, space="PSUM"))
        ps1 = ctx.enter_context(tc.tile_pool(name="ps1", bufs=1, space="PSUM"))
        sb2 = ctx.enter_context(tc.tile_pool(name="sb2", bufs=6))

        # ---- input loads -------------------------------------------------
        obs_view = obs_h.ap().rearrange("(j p) s -> p j s", p=P)
        obs_i = sb.tile([P, J, N_SPACE], i32)
        for q in range(8):
            eng = nc.sync if q % 2 == 0 else nc.scalar
            eng.dma_start(obs_i[:, 4 * q:4 * q + 4, :],
                          obs_view[:, 4 * q:4 * q + 4, :])

        beta_bc = sb.tile([P, 1], f32)
        nc.scalar.dma_start(beta_bc[:], bass.AP(beta_h, 0, [[0, P], [1, 1]]))

        adiag = sb.tile([1, N_SPACE], f32)
        nc.scalar.dma_start(adiag[:], bass.AP(alpha_h, 0, [[0, 1], [N_SPACE + 1, N_SPACE]]))

        rhs2 = sb.tile([2, J * N_SPACE], bf16)  # row0 = carry C flat, row1 = mu tiled
        mu_f = sb.tile([1, N_SPACE], f32)
        nc.scalar.dma_start(mu_f[:], bass.AP(mu_h, 0, [[0, 1], [1, N_SPACE]]))
        mu_b = sb.tile([1, N_SPACE], bf16)
        nc.vector.tensor_copy(mu_b[:], mu_f[:])
        nc.scalar.dma_start(
            rhs2[1:2, :].rearrange("o (j s) -> o j s", s=N_SPACE),
            mu_b[:].unsqueeze(1).broadcast_to((1, J, N_SPACE)))

        idx = sb.tile([P, NSP], i32)
        nc.scalar.dma_start(idx[:], bass.AP(gidx_h, 0, [[NSP, P], [1, NSP]]))
        phiw = sb.tile([P, NCH * NWIN], i32)
        nc.scalar.dma_start(phiw[:], phiw_h.ap())

        # ---- runtime constants from beta --------------------------------
        negb = sb.tile([P, 1], f32)
        nc.vector.tensor_scalar(out=negb[:], in0=beta_bc[:], scalar1=-1.0,
                                scalar2=None, op0=Alu.mult)
        negb128 = sb.tile([P, 1], f32)
        nc.vector.tensor_scalar(out=negb128[:], in0=negb[:], scalar1=128.0,
                                scalar2=None, op0=Alu.mult)

        # LdT[tp, m] = exp(-b (m - tp)) for tp < m else 0   (within-block decay)
        xd = sb.tile([P, P], i32)
        nc.gpsimd.iota(xd[:], [[1, P]], base=0, channel_multiplier=-1)   # f - p
        lda = sb.tile([P, P], f32)
        nc.vector.tensor_scalar(out=lda[:], in0=xd[:], scalar1=negb[:],
                                scalar2=None, op0=Alu.mult)
        ldb = sb.tile([P, P], f32)
        nc.vector.tensor_scalar(out=ldb[:], in0=xd[:], scalar1=1000.0,
                                scalar2=-1000.0, op0=Alu.mult, op1=Alu.add)
        ldm = sb.tile([P, P], f32)
        nc.vector.tensor_tensor(out=ldm[:], in0=lda[:], in1=ldb[:], op=Alu.min)
        ldt = sb.tile([P, P], f32)
        nc.scalar.activation(ldt[:], ldm[:], Act.Exp)
        ldtb = sb.tile([P, P], bf16)
        nc.vector.tensor_copy(ldtb[:], ldt[:])

        # v[tp] = exp(-b (128 - tp))  (end-of-block carry weights)
        xv = sb.tile([P, 1], i32)
        nc.gpsimd.iota(xv[:], [[0, 1]], base=P, channel_multiplier=-1)   # 128 - p
        vm = sb.tile([P, 1], f32)
        nc.vector.tensor_scalar(out=vm[:], in0=xv[:], scalar1=negb[:],
                                scalar2=None, op0=Alu.mult)
        vv = sb.tile([P, 1], f32)
        nc.scalar.activation(vv[:], vm[:], Act.Exp)
        vvb = sb.tile([P, 1], bf16)
        nc.vector.tensor_copy(vvb[:], vv[:])

        # LcT[k, j] = exp(-128 b (j - 1 - k)) for k <= j-1 else 0  (carry matrix)
        xc = sb.tile([J, J], i32)
        nc.gpsimd.iota(xc[:], [[1, J]], base=-1, channel_multiplier=-1)  # f - 1 - p
        lca = sb.tile([J, J], f32)
        nc.vector.tensor_scalar(out=lca[:], in0=xc[:], scalar1=negb128[:J, :],
                                scalar2=None, op0=Alu.mult)
        lcb = sb.tile([J, J], f32)
        nc.vector.tensor_scalar(out=lcb[:], in0=xc[:], scalar1=1000.0,
                                scalar2=None, op0=Alu.mult)
        lcm = sb.tile([J, J], f32)
        nc.vector.tensor_tensor(out=lcm[:], in0=lca[:], in1=lcb[:], op=Alu.min)
        lct = sb.tile([J, J], f32)
        nc.scalar.activation(lct[:], lcm[:], Act.Exp)

        # u2: row0 = u_i = exp(-b i), row1 = ones (mu term).
        # scale vector [-b; 0] makes exp produce both rows at once.
        negb01 = sb.tile([2, 1], f32)
        nc.vector.memset(negb01[:], 0.0)
        nc.vector.tensor_copy(negb01[0:1, :], negb[0:1, :])
        xu = sb.tile([2, P], i32)
        nc.gpsimd.iota(xu[:], [[1, P]], base=0, channel_multiplier=0)    # f
        um = sb.tile([2, P], f32)
        nc.vector.tensor_scalar(out=um[:], in0=xu[:], scalar1=negb01[:],
                                scalar2=None, op0=Alu.mult)
        u2 = sb.tile([2, P], f32)
        nc.scalar.activation(u2[:], um[:], Act.Exp)
        u2b = sb.tile([2, P], bf16)
        nc.vector.tensor_copy(u2b[:], u2[:])

        # asb[s] = b * alpha[s, s], broadcast to all 128 partitions via PE
        asb_row = sb.tile([1, N_SPACE], f32)
        nc.vector.tensor_scalar(out=asb_row[:], in0=adiag[:],
                                scalar1=beta_bc[:1, :], scalar2=None, op0=Alu.mult)
        ones1 = sb.tile([1, P], f32)
        nc.vector.memset(ones1[:], 1.0)
        asb_ps = ps1.tile([P, N_SPACE], f32)
        nc.tensor.matmul(asb_ps[:], lhsT=ones1[:], rhs=asb_row[:], start=True, stop=True)
        asb_bc = sb.tile([P, N_SPACE], f32)
        nc.vector.tensor_copy(asb_bc[:], asb_ps[:])

        # one-hot window masks for every (chunk, window) slot, built in
        # per-quarter slices interleaved with obs_f so the first chunk's
        # matmul inputs are ready as early as possible:
        # mask[p, c, w, e] = (e == phiw[p, c*8+w])
        iota64 = sb.tile([P, EWIN], i32)
        nc.gpsimd.iota(iota64[:], [[1, EWIN]], base=0, channel_multiplier=0)
        masks = sb.tile([P, NCH * NWIN, EWIN], bf16)
        MQ = NCH * NWIN // 4      # mask columns per quarter slice

        def build_masks(q):
            nc.vector.tensor_tensor(
                out=masks[:, MQ * q:MQ * (q + 1), :],
                in0=iota64[:].unsqueeze(1).broadcast_to((P, MQ, EWIN)),
                in1=phiw[:, MQ * q:MQ * (q + 1)].unsqueeze(2)
                    .broadcast_to((P, MQ, EWIN)),
                op=Alu.is_equal)

        # obs_f[tp, j, s] = obs * asb[s]   (convert + scale, 4 chunked DVE passes)
        obs_f = sb.tile([P, J * N_SPACE], bf16)
        obs_ff = obs_f[:]                # [P, 8192] flat view
        obs_f3 = obs_f[:].rearrange("p (j s) -> p j s", s=N_SPACE)
        for q in range(4):
            nc.vector.tensor_tensor(
                out=obs_f3[:, 8 * q:8 * q + 8, :],
                in0=obs_i[:, 8 * q:8 * q + 8, :],
                in1=asb_bc[:].unsqueeze(1).broadcast_to((P, 8, N_SPACE)),
                op=Alu.mult,
            )
            build_masks(q)

        # ---- fused build + per-chunk gather pipeline --------------------
        r_flat = sb.tile([1, J * N_SPACE], f32)
        r32 = sb.tile([J, N_SPACE], f32)
        rhs2_j = rhs2[0:1, :].rearrange("o (j s) -> o j s", s=N_SPACE)
        g_store = bass.AP(g_h, 0, [[N_SPACE, P], [P * N_SPACE, J], [1, N_SPACE]])

        gath = sb.tile([P, 2 * NSP], f32)
        views = [bass.AP(g_h, 0, [[1, min(256 * (b + 1), N_TIME) * N_SPACE
                                   + (2 if b == 15 else 0)], [1, 1]])
                 for b in range(16)]
        zpad = sb.tile([1, 2], f32)
        nc.vector.memset(zpad[:], 0.0)
        nc.sync.dma_start(bass.AP(g_h, N_TIME * N_SPACE, [[1, 1], [1, 2]]), zpad[:])

        lam_all = sb.tile([P, NCH * NWIN], f32)   # DVE-served results
        lam_sp = sb.tile([P, NSP], f32)           # spill results

        def fire_spill(f):
            nc.gpsimd.indirect_dma_start(
                out=gath[:, 2 * f:2 * f + 2],
                out_offset=None,
                in_=views[SP_BOUND[f]],
                in_offset=bass.IndirectOffsetOnAxis(ap=idx[:, f:f + 1], axis=0),
            )
            if f % 4 == 3:
                g = f // 4
                nc.vector.tensor_scalar(
                    out=lam_sp[:, 4 * g:4 * g + 4].rearrange(
                        "p (f o) -> p f o", o=1),
                    in0=gath[:].rearrange("p (f o) -> p f o", o=2)[
                        :, 4 * g:4 * g + 4, 0:1],
                    scalar1=float(LAM_MIN), scalar2=None, op0=Alu.max)

        def emit_r(c):
            r_ps = psr.tile([1, CH], f32)
            nc.tensor.matmul(r_ps[:], lhsT=vvb[:],
                             rhs=obs_ff[:, c * CH:(c + 1) * CH],
                             start=True, stop=True)
            nc.scalar.activation(r_flat[:, c * CH:(c + 1) * CH], r_ps[:],
                                 Act.Copy)

        def emit_gchunk(c):
            pch = ps.tile([P, CH], f32)
            nc.tensor.matmul(pch[:], lhsT=ldtb[:],
                             rhs=obs_ff[:, c * CH:(c + 1) * CH],
                             start=True, stop=True)
            nc.tensor.matmul(pch[:], lhsT=u2b[:],
                             rhs=rhs2[:, c * CH:(c + 1) * CH],
                             start=False, stop=True, skip_group_check=True)
            gch = sb2.tile([P, CH], f32, tag="gch")
            nc.scalar.activation(gch[:], pch[:], Act.Copy)
            jj = c * CH // N_SPACE
            eng = nc.sync if c % 2 == 0 else nc.scalar
            eng.dma_start(g_store[:, jj:jj + CH // N_SPACE, :], gch[:])
            # primary gather: one-hot select per (window, partition) cell
            # from the SBUF chunk tile (mult + reduce + clip on DVE).
            prod = sb2.tile([P, NWIN, EWIN], bf16, tag="prod")
            nc.vector.tensor_tensor(
                out=prod[:],
                in0=pch[:].rearrange("p (w e) -> p w e", e=EWIN),
                in1=masks[:, c * NWIN:(c + 1) * NWIN, :],
                op=Alu.mult)
            val = sb2.tile([P, NWIN], f32, tag="val")
            nc.vector.tensor_reduce(out=val[:], in_=prod[:],
                                    axis=mybir.AxisListType.X, op=Alu.add)
            nc.vector.tensor_scalar(out=lam_all[:, c * NWIN:(c + 1) * NWIN],
                                    in0=val[:], scalar1=float(LAM_MIN),
                                    scalar2=None, op0=Alu.max)
            # spill columns whose t-bound chunk just landed in DRAM.
            for f in range(NSP):
                if SP_BOUND[f] == c:
                    fire_spill(f)

        def emit_carry(k):
            c_ps = ps1.tile([8, N_SPACE], f32, tag="cps")
            nc.tensor.matmul(c_ps[:], lhsT=lct[0:8 * (k + 1), 8 * k:8 * (k + 1)],
                             rhs=r32[0:8 * (k + 1), :], start=True, stop=True)
            c32 = sb2.tile([8, N_SPACE], bf16, tag="c32")
            nc.vector.tensor_copy(c32[:], c_ps[:])
            nc.sync.dma_start(rhs2_j[:, 8 * k:8 * k + 8, :], c32[:])

        for k in range(4):
            if k == 0:
                # fast path for chunk 0: its carry rows are C[0] = 0 and
                # C[1] = z[0] (= r of chunk 0, first half), so it can build,
                # store, and start gathering before the quarter carry chain.
                emit_r(0)
                nc.vector.memset(rhs2_j[:, 0:1, :], 0.0)
                nc.vector.tensor_copy(rhs2_j[:, 1:2, :],
                                      r_flat[:, 0:N_SPACE].unsqueeze(1))
                emit_gchunk(0)
                for c in range(1, 4):
                    emit_r(c)
            else:
                pass  # this quarter's r matmuls were hoisted a quarter early
            nc.sync.dma_start(r32[8 * k:8 * k + 8, :],
                              r_flat[:, 2048 * k:2048 * (k + 1)])
            emit_carry(k)
            if k < 3:
                for c in range(4 * (k + 1), 4 * (k + 1) + 4):
                    emit_r(c)
            for c in range(4 * k + (1 if k == 0 else 0), 4 * k + 4):
                emit_gchunk(c)
        nc.sync.dma_start(
            bass.AP(out_h, 0, [[FQ2, P], [1, NCH * NWIN]]), lam_all[:])
        nc.sync.dma_start(
            bass.AP(out_h, NCH * NWIN, [[FQ2, P], [1, NSP]]), lam_sp[:])

    nc.compile()
    return nc


_NC_CACHE = None


def _get_nc():
    global _NC_CACHE
    if _NC_CACHE is None:
        _NC_CACHE = build_nc()
    return _NC_CACHE


def _route_queries(tc_, sc_):
    """Route one core's queries.

    Primary: query (t, s) belongs to chunk c = t >> 8, window
    w = ((t >> 7) & 1)*4 + (s >> 6), partition p = t % 128, and in-window
    offset s & 63.  The first query of each (c, w, p) cell gets the DVE
    one-hot slot -> out flat c*1024 + w*128 + p.  Overflow spills, sorted
    by t, into NSP indirect-gather columns (column f only holds
    t < 256*(SP_BOUND[f]+1)); spill slot (f, p) -> out flat
    SPILL0 + f*128 + p.

    Returns (gidx [NSP*128] i32, phiw [128, 128] i32,
    (dev_pos, orig_pos)).
    """
    n = tc_.shape[0]
    t64 = tc_.astype(np.int64)
    s64 = sc_.astype(np.int64)
    chunk = t64 >> 8
    win = ((t64 >> 7) & 1) * (NWIN // 2) + (s64 >> 6)
    part = t64 % P
    flat_all = t64 * N_SPACE + s64

    phiw = np.full((P, NCH * NWIN), -1, np.int32)
    gidx = np.zeros((P, NSP), np.int32)
    dev_pos = np.empty(n, np.int64)

    # rank within (chunk, window, partition) cell, in input order
    cellid = (chunk * NWIN + win) * P + part
    order = np.argsort(cellid, kind="stable")
    oc = cellid[order]
    starts = np.r_[0, np.flatnonzero(np.diff(oc)) + 1]
    rank = np.empty(n, np.int64)
    lens = np.diff(np.r_[starts, n])
    rank[order] = np.concatenate([np.arange(l) for l in lens])

    pi = np.flatnonzero(rank == 0)
    phiw[part[pi], chunk[pi] * NWIN + win[pi]] = s64[pi] & (EWIN - 1)
    dev_pos[pi] = part[pi] * FQ2 + chunk[pi] * NWIN + win[pi]

    # spill: sorted by t, greedy bounded fill
    si = np.flatnonzero(rank > 0)
    si = si[np.argsort(t64[si], kind="stable")]
    sts = t64[si]
    lo = 0
    for f in range(NSP):
        bound = 256 * (SP_BOUND[f] + 1)
        hi = np.searchsorted(sts, bound, side="left")
        take = min(P, hi - lo) if f < NSP - 1 else (len(si) - lo)
        if take > P:
            raise RuntimeError("spill overflow: t-distribution infeasible")
        sel = si[lo:lo + take]
        pp = np.arange(take)
        gidx[pp, f] = flat_all[sel]
        dev_pos[sel] = pp * FQ2 + NCH * NWIN + f
        lo += take
    orig_pos = np.arange(n)
    return gidx.reshape(-1), phiw, (dev_pos, orig_pos)


def _make_in_maps(t, s, obs, mu, alpha, beta):
    in_maps, perms = [], []
    for c in range(N_CORES):
        sl = slice(c * PER_CORE, (c + 1) * PER_CORE)
        gidx, phiw, perm = _route_queries(t[sl], s[sl])
        perms.append(perm)
        in_maps.append({
            "gidx": gidx, "phiw": phiw,
            "obs": obs, "mu": mu, "alpha": alpha, "beta": beta,
        })
    return in_maps, perms


def kernel(t, s, obs, mu, alpha, beta, **_unused):
    t = np.ascontiguousarray(np.asarray(t, dtype=np.int32))
    s = np.ascontiguousarray(np.asarray(s, dtype=np.int32))
    obs = np.ascontiguousarray(np.asarray(obs, dtype=np.int32))
    mu = np.ascontiguousarray(np.asarray(mu, dtype=np.float32))
    alpha = np.ascontiguousarray(np.asarray(alpha, dtype=np.float32))
    beta = np.ascontiguousarray(np.asarray(beta, dtype=np.float32))

    nc = _get_nc()
    in_maps, perms = _make_in_maps(t, s, obs, mu, alpha, beta)
    res = run_bass_kernel_spmd(nc, in_maps, core_ids=list(range(N_CORES)))
    outs = []
    for c in range(N_CORES):
        dev = res.results[c]["out"]          # [NSLOT]
        o = np.empty(PER_CORE, np.float32)
        o[perms[c][1]] = dev[perms[c][0]]
        outs.append(o)
    return np.concatenate(outs).astype(np.float32)


if __name__ == "__main__":
    # quick self-check against a numpy re-implementation on random data
    rng = np.random.default_rng(0)
    t = rng.integers(0, N_TIME, BATCH).astype(np.int32)
    s = rng.integers(0, N_SPACE, BATCH).astype(np.int32)
    obs = rng.integers(0, 10, (N_TIME, N_SPACE)).astype(np.int32)
    mu = rng.random(N_SPACE, dtype=np.float32)
    alpha = rng.random((N_SPACE, N_SPACE), dtype=np.float32)
    beta = (rng.random(1, dtype=np.float32) + 0.1).astype(np.float32)

    got = kernel(t=t, s=s, obs=obs, mu=mu, alpha=alpha, beta=beta)

    b = float(beta[0])
    e = np.exp(-b)
    F = np.zeros((N_TIME, N_SPACE), np.float64)
    for tt in range(1, N_TIME):
        F[tt] = e * (F[tt - 1] + obs[tt - 1])
    G = np.clip(mu[None, :] + np.diag(alpha)[None, :] * b * F, LAM_MIN, None)
    want = G[t, s].astype(np.float32)
    err = np.abs(got - want) / np.maximum(np.abs(want), 1e-6)
    print("max rel err:", err.max(), "mean:", err.mean())


# revision 42
# speedup vs baseline: 1.0864x; 1.0864x over previous
"""Discrete Hawkes conditional-intensity kernel for 8 Trainium2 NeuronCores.

Math
----
Reference computes, per query i with (t, s) = (t_i, s_i):

    lam_i = clip(mu[s] + alpha[s, s] * b * F[t, s], 1e-5)
    F[t, s] = sum_{tp < t} obs[tp, s] * exp(-b * (t - tp))

F obeys F[t+1] = e * (F[t] + obs[t]), e = exp(-b), i.e. it is an
exponentially-decayed prefix sum over time.  On device we build the full
table G[t, s] = mu[s] + alpha[s,s]*b*F[t, s] with a blocked formulation
(time blocks of 128 on the PE array + a 32-step cross-block carry), store
it to DRAM, and answer the 8192 queries per core with a split gather
(vector-engine one-hot selects from SBUF + indirect-DMA spill).

The gather is the crux: every DMA-based path for data-dependent
addressing costs ~1us of serial GPSIMD per 128 elements (SWDGE fixed
overhead; dma_gather ucode ~7ns/desc, ap_gather ~27ns/idx are no
better), which walls a pure indirect-DMA gather at ~70us/core.  So most
queries never touch a DMA: each G chunk tile [128, 512] is split into 8
windows of 64, and the first query of every (chunk, window, partition)
cell (~6430 of 8192) is answered on the vector engine by a prebuilt
one-hot mask + multiply + window-reduce straight from SBUF, pipelined
behind the chunk's matmuls.  Only cell-overflow queries (~1780) use
staged indirect-DMA columns against the DRAM copy of G.  Output slots
are partition-major so the result stores are contiguous per partition
(a transposed store pattern costs 16K four-byte descriptors, ~45us).

Sharding: queries (t, s) are split 8x8192 across cores (data parallel);
obs / mu / alpha / beta are replicated.  No collectives needed.
"""

import os
import sys

import numpy as np

_REPO_CANDIDATES = ("/opt/trn_rl_repo", os.path.expanduser("~/.axon_site/_ro/trn_rl_repo"))
for _p in _REPO_CANDIDATES:
    if os.path.isdir(_p) and _p not in sys.path:
        sys.path.append(_p)

import concourse.bass as bass
import concourse.tile as tile
from concourse import bacc, mybir
from concourse.bass_utils import run_bass_kernel_spmd

# Problem constants (hardcoded per spec).
N_TIME = 4096
N_SPACE = 256
BATCH = 65536
N_CORES = 8
LAM_MIN = 1e-5

P = 128               # partitions / time-block size
J = N_TIME // P       # 32 time blocks
PER_CORE = BATCH // N_CORES   # 8192 queries per core
CH = 512              # matmul N-chunk (one PSUM bank)
NCH = (J * N_SPACE) // CH     # 16 chunks over the (j, s) flat axis

# Gather layout.  Primary path: the G chunk tile [128, 512] is split into
# 8 windows of 64; each (chunk, window, partition) cell serves at most ONE
# query via a one-hot mask + multiply + reduce on the vector engine,
# straight from SBUF (~160 scanned elems per served query vs ~1.1us of
# serial GPSIMD per 128 queries for indirect DMA).  The one-hot masks for
# all 16 chunks are prebuilt in a single DVE op during the obs load.
# Cells with 2+ queries (~1780 of 8192) spill to NSP indirect-DMA columns
# gathered from the DRAM copy of G, staged by t so they fire as the table
# lands.
NWIN = 8              # 64-elem windows per chunk tile
EWIN = CH // NWIN     # 64
NSP = 16              # spill columns (128 queries each)
FQ2 = NCH * NWIN + NSP          # 144 output columns per partition
NSLOT = P * FQ2                 # 18432 slots; flat = p*FQ2 + col
# spill column f may only hold queries with t < 256*(SP_BOUND[f]+1); one
# chunk of lookahead keeps the sorted greedy fill from starving.
SP_BOUND = [min(15, f + 1) for f in range(NSP)]

f32 = mybir.dt.float32
bf16 = mybir.dt.bfloat16
i32 = mybir.dt.int32
Alu = mybir.AluOpType
Act = mybir.ActivationFunctionType


def build_nc():
    nc = bacc.Bacc("TRN2", target_bir_lowering=False, debug=False)

    gidx_h = nc.dram_tensor("gidx", [NSP * P], i32, kind="ExternalInput")
    phiw_h = nc.dram_tensor("phiw", [P, NCH * NWIN], i32, kind="ExternalInput")
    obs_h = nc.dram_tensor("obs", [N_TIME, N_SPACE], i32, kind="ExternalInput")
    mu_h = nc.dram_tensor("mu", [N_SPACE], f32, kind="ExternalInput")
    alpha_h = nc.dram_tensor("alpha", [N_SPACE, N_SPACE], f32, kind="ExternalInput")
    beta_h = nc.dram_tensor("beta", [1], f32, kind="ExternalInput")
    g_h = nc.dram_tensor("gtab", [N_TIME * N_SPACE + 2], f32, kind="Internal")
    out_h = nc.dram_tensor("out", [NSLOT], f32, kind="ExternalOutput")

    from contextlib import ExitStack

    with tile.TileContext(nc) as tc, ExitStack() as ctx:
        sb = ctx.enter_context(tc.tile_pool(name="sb", bufs=1))
        ps = ctx.enter_context(tc.tile_pool(name="ps", bufs=4, space="PSUM"))
        psr = ctx.enter_context(tc.tile_pool(name="psr", bufs=2, space="PSUM"))
        ps1 = ctx.enter_context(tc.tile_pool(name="ps1", bufs=1, space="PSUM"))
        sb2 = ctx.enter_context(tc.tile_pool(name="sb2", bufs=6))

        # ---- input loads -------------------------------------------------
        beta_bc = sb.tile([P, 1], f32)
        nc.scalar.dma_start(beta_bc[:], bass.AP(beta_h, 0, [[0, P], [1, 1]]))

        adiag = sb.tile([1, N_SPACE], f32)
        nc.scalar.dma_start(adiag[:], bass.AP(alpha_h, 0, [[0, 1], [N_SPACE + 1, N_SPACE]]))

        rhs2 = sb.tile([2, J * N_SPACE], bf16)  # row0 = carry C flat, row1 = mu tiled
        mu_f = sb.tile([1, N_SPACE], f32)
        nc.scalar.dma_start(mu_f[:], bass.AP(mu_h, 0, [[0, 1], [1, N_SPACE]]))
        mu_b = sb.tile([1, N_SPACE], bf16)
        nc.vector.tensor_copy(mu_b[:], mu_f[:])
        nc.scalar.dma_start(
            rhs2[1:2, :].rearrange("o (j s) -> o j s", s=N_SPACE),
            mu_b[:].unsqueeze(1).broadcast_to((1, J, N_SPACE)))

        idx = sb.tile([P, NSP], i32)
        nc.scalar.dma_start(idx[:], bass.AP(gidx_h, 0, [[NSP, P], [1, NSP]]))
        phiw = sb.tile([P, NCH * NWIN], i32)
        nc.scalar.dma_start(phiw[:], phiw_h.ap())

        # obs after the small loads: split across both HWDGE queues, small
        # gating loads stay at the queue heads.
        obs_view = obs_h.ap().rearrange("(j p) s -> p j s", p=P)
        obs_i = sb.tile([P, J, N_SPACE], i32)
        for q in range(8):
            eng = nc.sync if q % 2 == 0 else nc.scalar
            eng.dma_start(obs_i[:, 4 * q:4 * q + 4, :],
                          obs_view[:, 4 * q:4 * q + 4, :])

        # ---- runtime constants from beta --------------------------------
        negb = sb.tile([P, 1], f32)
        nc.vector.tensor_scalar(out=negb[:], in0=beta_bc[:], scalar1=-1.0,
                                scalar2=None, op0=Alu.mult)
        negb128 = sb.tile([P, 1], f32)
        nc.vector.tensor_scalar(out=negb128[:], in0=negb[:], scalar1=128.0,
                                scalar2=None, op0=Alu.mult)

        # LdT[tp, m] = exp(-b (m - tp)) for tp < m else 0   (within-block decay)
        xd = sb.tile([P, P], i32)
        nc.gpsimd.iota(xd[:], [[1, P]], base=0, channel_multiplier=-1)   # f - p
        lda = sb.tile([P, P], f32)
        nc.vector.tensor_scalar(out=lda[:], in0=xd[:], scalar1=negb[:],
                                scalar2=None, op0=Alu.mult)
        ldb = sb.tile([P, P], f32)
        nc.vector.tensor_scalar(out=ldb[:], in0=xd[:], scalar1=1000.0,
                                scalar2=-1000.0, op0=Alu.mult, op1=Alu.add)
        ldm = sb.tile([P, P], f32)
        nc.vector.tensor_tensor(out=ldm[:], in0=lda[:], in1=ldb[:], op=Alu.min)
        ldt = sb.tile([P, P], f32)
        nc.scalar.activation(ldt[:], ldm[:], Act.Exp)
        ldtb = sb.tile([P, P], bf16)
        nc.vector.tensor_copy(ldtb[:], ldt[:])

        # v[tp] = exp(-b (128 - tp))  (end-of-block carry weights)
        xv = sb.tile([P, 1], i32)
        nc.gpsimd.iota(xv[:], [[0, 1]], base=P, channel_multiplier=-1)   # 128 - p
        vm = sb.tile([P, 1], f32)
        nc.vector.tensor_scalar(out=vm[:], in0=xv[:], scalar1=negb[:],
                                scalar2=None, op0=Alu.mult)
        vv = sb.tile([P, 1], f32)
        nc.scalar.activation(vv[:], vm[:], Act.Exp)
        vvb = sb.tile([P, 1], bf16)
        nc.vector.tensor_copy(vvb[:], vv[:])

        # LcT[k, j] = exp(-128 b (j - 1 - k)) for k <= j-1 else 0  (carry matrix)
        xc = sb.tile([J, J], i32)
        nc.gpsimd.iota(xc[:], [[1, J]], base=-1, channel_multiplier=-1)  # f - 1 - p
        lca = sb.tile([J, J], f32)
        nc.vector.tensor_scalar(out=lca[:], in0=xc[:], scalar1=negb128[:J, :],
                                scalar2=None, op0=Alu.mult)
        lcb = sb.tile([J, J], f32)
        nc.vector.tensor_scalar(out=lcb[:], in0=xc[:], scalar1=1000.0,
                                scalar2=None, op0=Alu.mult)
        lcm = sb.tile([J, J], f32)
        nc.vector.tensor_tensor(out=lcm[:], in0=lca[:], in1=lcb[:], op=Alu.min)
        lct = sb.tile([J, J], f32)
        nc.scalar.activation(lct[:], lcm[:], Act.Exp)

        # u2: row0 = u_i = exp(-b i), row1 = ones (mu term).
        # scale vector [-b; 0] makes exp produce both rows at once.
        negb01 = sb.tile([2, 1], f32)
        nc.vector.memset(negb01[:], 0.0)
        nc.vector.tensor_copy(negb01[0:1, :], negb[0:1, :])
        xu = sb.tile([2, P], i32)
        nc.gpsimd.iota(xu[:], [[1, P]], base=0, channel_multiplier=0)    # f
        um = sb.tile([2, P], f32)
        nc.vector.tensor_scalar(out=um[:], in0=xu[:], scalar1=negb01[:],
                                scalar2=None, op0=Alu.mult)
        u2 = sb.tile([2, P], f32)
        nc.scalar.activation(u2[:], um[:], Act.Exp)
        u2b = sb.tile([2, P], bf16)
        nc.vector.tensor_copy(u2b[:], u2[:])

        # asb[s] = b * alpha[s, s], broadcast to all 128 partitions via PE
        asb_row = sb.tile([1, N_SPACE], f32)
        nc.vector.tensor_scalar(out=asb_row[:], in0=adiag[:],
                                scalar1=beta_bc[:1, :], scalar2=None, op0=Alu.mult)
        ones1 = sb.tile([1, P], f32)
        nc.vector.memset(ones1[:], 1.0)
        asb_ps = ps1.tile([P, N_SPACE], f32)
        nc.tensor.matmul(asb_ps[:], lhsT=ones1[:], rhs=asb_row[:], start=True, stop=True)
        asb_bc = sb.tile([P, N_SPACE], f32)
        nc.vector.tensor_copy(asb_bc[:], asb_ps[:])

        # one-hot window masks for every (chunk, window) slot, built in
        # per-quarter slices interleaved with obs_f so the first chunk's
        # matmul inputs are ready as early as possible:
        # mask[p, c, w, e] = (e == phiw[p, c*8+w])
        iota64 = sb.tile([P, EWIN], i32)
        nc.gpsimd.iota(iota64[:], [[1, EWIN]], base=0, channel_multiplier=0)
        masks = sb.tile([P, NCH * NWIN, EWIN], bf16)
        MQ = NCH * NWIN // 4      # mask columns per quarter slice

        def build_masks(q):
            nc.vector.tensor_tensor(
                out=masks[:, MQ * q:MQ * (q + 1), :],
                in0=iota64[:].unsqueeze(1).broadcast_to((P, MQ, EWIN)),
                in1=phiw[:, MQ * q:MQ * (q + 1)].unsqueeze(2)
                    .broadcast_to((P, MQ, EWIN)),
                op=Alu.is_equal)

        # obs_f[tp, j, s] = obs * asb[s]   (convert + scale, 4 chunked DVE passes)
        obs_f = sb.tile([P, J * N_SPACE], bf16)
        obs_ff = obs_f[:]                # [P, 8192] flat view
        obs_f3 = obs_f[:].rearrange("p (j s) -> p j s", s=N_SPACE)
        for q in range(4):
            nc.vector.tensor_tensor(
                out=obs_f3[:, 8 * q:8 * q + 8, :],
                in0=obs_i[:, 8 * q:8 * q + 8, :],
                in1=asb_bc[:].unsqueeze(1).broadcast_to((P, 8, N_SPACE)),
                op=Alu.mult,
            )
            build_masks(q)

        # ---- fused build + per-chunk gather pipeline --------------------
        r_flat = sb.tile([1, J * N_SPACE], f32)
        r32 = sb.tile([J, N_SPACE], f32)
        rhs2_j = rhs2[0:1, :].rearrange("o (j s) -> o j s", s=N_SPACE)
        g_store = bass.AP(g_h, 0, [[N_SPACE, P], [P * N_SPACE, J], [1, N_SPACE]])

        gath = sb.tile([P, 2 * NSP], f32)
        views = [bass.AP(g_h, 0, [[1, min(256 * (b + 1), N_TIME) * N_SPACE
                                   + (2 if b == 15 else 0)], [1, 1]])
                 for b in range(16)]
        zpad = sb.tile([1, 2], f32)
        nc.vector.memset(zpad[:], 0.0)
        nc.sync.dma_start(bass.AP(g_h, N_TIME * N_SPACE, [[1, 1], [1, 2]]), zpad[:])

        lam_all = sb.tile([P, NCH * NWIN], f32)   # DVE-served results
        lam_sp = sb.tile([P, NSP], f32)           # spill results

        def fire_spill(f):
            nc.gpsimd.indirect_dma_start(
                out=gath[:, 2 * f:2 * f + 2],
                out_offset=None,
                in_=views[SP_BOUND[f]],
                in_offset=bass.IndirectOffsetOnAxis(ap=idx[:, f:f + 1], axis=0),
            )
            if f % 4 == 3:
                g = f // 4
                nc.vector.tensor_scalar(
                    out=lam_sp[:, 4 * g:4 * g + 4].rearrange(
                        "p (f o) -> p f o", o=1),
                    in0=gath[:].rearrange("p (f o) -> p f o", o=2)[
                        :, 4 * g:4 * g + 4, 0:1],
                    scalar1=float(LAM_MIN), scalar2=None, op0=Alu.max)

        def emit_r(c):
            r_ps = psr.tile([1, CH], f32)
            nc.tensor.matmul(r_ps[:], lhsT=vvb[:],
                             rhs=obs_ff[:, c * CH:(c + 1) * CH],
                             start=True, stop=True)
            nc.scalar.activation(r_flat[:, c * CH:(c + 1) * CH], r_ps[:],
                                 Act.Copy)

        def emit_gchunk(c):
            pch = ps.tile([P, CH], f32)
            nc.tensor.matmul(pch[:], lhsT=ldtb[:],
                             rhs=obs_ff[:, c * CH:(c + 1) * CH],
                             start=True, stop=True)
            nc.tensor.matmul(pch[:], lhsT=u2b[:],
                             rhs=rhs2[:, c * CH:(c + 1) * CH],
                             start=False, stop=True, skip_group_check=True)
            gch = sb2.tile([P, CH], f32, tag="gch")
            nc.scalar.activation(gch[:], pch[:], Act.Copy)
            jj = c * CH // N_SPACE
            eng = nc.sync if c % 2 == 0 else nc.scalar
            eng.dma_start(g_store[:, jj:jj + CH // N_SPACE, :], gch[:])
            # primary gather: one-hot select per (window, partition) cell
            # from the SBUF chunk tile (mult + reduce + clip on DVE).
            prod = sb2.tile([P, NWIN, EWIN], bf16, tag="prod")
            nc.vector.tensor_tensor(
                out=prod[:],
                in0=pch[:].rearrange("p (w e) -> p w e", e=EWIN),
                in1=masks[:, c * NWIN:(c + 1) * NWIN, :],
                op=Alu.mult)
            val = sb2.tile([P, NWIN], f32, tag="val")
            nc.vector.tensor_reduce(out=val[:], in_=prod[:],
                                    axis=mybir.AxisListType.X, op=Alu.add)
            nc.vector.tensor_scalar(out=lam_all[:, c * NWIN:(c + 1) * NWIN],
                                    in0=val[:], scalar1=float(LAM_MIN),
                                    scalar2=None, op0=Alu.max)
            # spill columns whose t-bound chunk just landed in DRAM.
            for f in range(NSP):
                if SP_BOUND[f] == c:
                    fire_spill(f)

        def emit_carry(k):
            c_ps = ps1.tile([8, N_SPACE], f32, tag="cps")
            nc.tensor.matmul(c_ps[:], lhsT=lct[0:8 * (k + 1), 8 * k:8 * (k + 1)],
                             rhs=r32[0:8 * (k + 1), :], start=True, stop=True)
            c32 = sb2.tile([8, N_SPACE], bf16, tag="c32")
            nc.vector.tensor_copy(c32[:], c_ps[:])
            nc.sync.dma_start(rhs2_j[:, 8 * k:8 * k + 8, :], c32[:])

        for k in range(4):
            if k == 0:
                # fast path for chunk 0: its carry rows are C[0] = 0 and
                # C[1] = z[0] (= r of chunk 0, first half), so it can build,
                # store, and start gathering before the quarter carry chain.
                emit_r(0)
                nc.vector.memset(rhs2_j[:, 0:1, :], 0.0)
                nc.vector.tensor_copy(rhs2_j[:, 1:2, :],
                                      r_flat[:, 0:N_SPACE].unsqueeze(1))
                emit_gchunk(0)
                for c in range(1, 4):
                    emit_r(c)
            else:
                pass  # this quarter's r matmuls were hoisted a quarter early
            nc.sync.dma_start(r32[8 * k:8 * k + 8, :],
                              r_flat[:, 2048 * k:2048 * (k + 1)])
            emit_carry(k)
            if k < 3:
                for c in range(4 * (k + 1), 4 * (k + 1) + 4):
                    emit_r(c)
            for c in range(4 * k + (1 if k == 0 else 0), 4 * k + 4):
                emit_gchunk(c)
        nc.sync.dma_start(
            bass.AP(out_h, 0, [[FQ2, P], [1, NCH * NWIN]]), lam_all[:])
        nc.sync.dma_start(
            bass.AP(out_h, NCH * NWIN, [[FQ2, P], [1, NSP]]), lam_sp[:])

    nc.compile()
    return nc


_NC_CACHE = None


def _get_nc():
    global _NC_CACHE
    if _NC_CACHE is None:
        _NC_CACHE = build_nc()
    return _NC_CACHE


def _route_queries(tc_, sc_):
    """Route one core's queries.

    Primary: query (t, s) belongs to chunk c = t >> 8, window
    w = ((t >> 7) & 1)*4 + (s >> 6), partition p = t % 128, and in-window
    offset s & 63.  The first query of each (c, w, p) cell gets the DVE
    one-hot slot -> out flat c*1024 + w*128 + p.  Overflow spills, sorted
    by t, into NSP indirect-gather columns (column f only holds
    t < 256*(SP_BOUND[f]+1)); spill slot (f, p) -> out flat
    SPILL0 + f*128 + p.

    Returns (gidx [NSP*128] i32, phiw [128, 128] i32,
    (dev_pos, orig_pos)).
    """
    n = tc_.shape[0]
    t64 = tc_.astype(np.int64)
    s64 = sc_.astype(np.int64)
    chunk = t64 >> 8
    win = ((t64 >> 7) & 1) * (NWIN // 2) + (s64 >> 6)
    part = t64 % P
    flat_all = t64 * N_SPACE + s64

    phiw = np.full((P, NCH * NWIN), -1, np.int32)
    gidx = np.zeros((P, NSP), np.int32)
    dev_pos = np.empty(n, np.int64)

    # rank within (chunk, window, partition) cell, in input order
    cellid = (chunk * NWIN + win) * P + part
    order = np.argsort(cellid, kind="stable")
    oc = cellid[order]
    starts = np.r_[0, np.flatnonzero(np.diff(oc)) + 1]
    rank = np.empty(n, np.int64)
    lens = np.diff(np.r_[starts, n])
    rank[order] = np.concatenate([np.arange(l) for l in lens])

    pi = np.flatnonzero(rank == 0)
    phiw[part[pi], chunk[pi] * NWIN + win[pi]] = s64[pi] & (EWIN - 1)
    dev_pos[pi] = part[pi] * FQ2 + chunk[pi] * NWIN + win[pi]

    # spill: sorted by t, greedy bounded fill
    si = np.flatnonzero(rank > 0)
    si = si[np.argsort(t64[si], kind="stable")]
    sts = t64[si]
    lo = 0
    for f in range(NSP):
        bound = 256 * (SP_BOUND[f] + 1)
        hi = np.searchsorted(sts, bound, side="left")
        take = min(P, hi - lo) if f < NSP - 1 else (len(si) - lo)
        if take > P:
            raise RuntimeError("spill overflow: t-distribution infeasible")
        sel = si[lo:lo + take]
        pp = np.arange(take)
        gidx[pp, f] = flat_all[sel]
        dev_pos[sel] = pp * FQ2 + NCH * NWIN + f
        lo += take
    orig_pos = np.arange(n)
    return gidx.reshape(-1), phiw, (dev_pos, orig_pos)


def _make_in_maps(t, s, obs, mu, alpha, beta):
    in_maps, perms = [], []
    for c in range(N_CORES):
        sl = slice(c * PER_CORE, (c + 1) * PER_CORE)
        gidx, phiw, perm = _route_queries(t[sl], s[sl])
        perms.append(perm)
        in_maps.append({
            "gidx": gidx, "phiw": phiw,
            "obs": obs, "mu": mu, "alpha": alpha, "beta": beta,
        })
    return in_maps, perms


def kernel(t, s, obs, mu, alpha, beta, **_unused):
    t = np.ascontiguousarray(np.asarray(t, dtype=np.int32))
    s = np.ascontiguousarray(np.asarray(s, dtype=np.int32))
    obs = np.ascontiguousarray(np.asarray(obs, dtype=np.int32))
    mu = np.ascontiguousarray(np.asarray(mu, dtype=np.float32))
    alpha = np.ascontiguousarray(np.asarray(alpha, dtype=np.float32))
    beta = np.ascontiguousarray(np.asarray(beta, dtype=np.float32))

    nc = _get_nc()
    in_maps, perms = _make_in_maps(t, s, obs, mu, alpha, beta)
    res = run_bass_kernel_spmd(nc, in_maps, core_ids=list(range(N_CORES)))
    outs = []
    for c in range(N_CORES):
        dev = res.results[c]["out"]          # [NSLOT]
        o = np.empty(PER_CORE, np.float32)
        o[perms[c][1]] = dev[perms[c][0]]
        outs.append(o)
    return np.concatenate(outs).astype(np.float32)


if __name__ == "__main__":
    # quick self-check against a numpy re-implementation on random data
    rng = np.random.default_rng(0)
    t = rng.integers(0, N_TIME, BATCH).astype(np.int32)
    s = rng.integers(0, N_SPACE, BATCH).astype(np.int32)
    obs = rng.integers(0, 10, (N_TIME, N_SPACE)).astype(np.int32)
    mu = rng.random(N_SPACE, dtype=np.float32)
    alpha = rng.random((N_SPACE, N_SPACE), dtype=np.float32)
    beta = (rng.random(1, dtype=np.float32) + 0.1).astype(np.float32)

    got = kernel(t=t, s=s, obs=obs, mu=mu, alpha=alpha, beta=beta)

    b = float(beta[0])
    e = np.exp(-b)
    F = np.zeros((N_TIME, N_SPACE), np.float64)
    for tt in range(1, N_TIME):
        F[tt] = e * (F[tt - 1] + obs[tt - 1])
    G = np.clip(mu[None, :] + np.diag(alpha)[None, :] * b * F, LAM_MIN, None)
    want = G[t, s].astype(np.float32)
    err = np.abs(got - want) / np.maximum(np.abs(want), 1e-6)
    print("max rel err:", err.max(), "mean:", err.mean())
